# revision 4
# baseline (speedup 1.0000x reference)
"""Mat2Twist Trainium2 kernel: batch of 3x3 rotation matrices -> twist vectors.

For each matrix R:  tr = trace(R); x = (tr-1)/2 = cos(theta)
  theta = arccos(x) = pi/2 - arctan(x / sqrt(1 - x^2))
  w = [R21-R12, R02-R20, R10-R01]   (unnormalized axis, |w| = 2 sin theta)
  out = theta * w / (2 sin theta) = (pi/4 - arctan(x*r)/2) * r * w,
        r = 1/sqrt(1-x^2) = exp(-0.5*ln(1-x^2)) = 1/sin(theta)

Data-parallel over 8 NeuronCores. The host pre-arranges each core's
shard tile-major/component-major: chunk ci covers MS[ci] matrices per
partition, and within a partition-row the 9 components are stored as
contiguous blocks in PERM order, so every on-chip vector op and every
DMA is unit-stride:
  w  = chunk[0:3m] - chunk[3m:6m]      (one fused subtract, 3m wide)
  tr = chunk[6m:7m]+chunk[7m:8m]+chunk[8m:9m]
Output is produced component-major per chunk and re-interleaved on host.

Perf structure (memory-bound problem, ~358 GB/s HBM per core):
  - input DMAs issue on the SP HWDGE ring (nc.sync), output DMAs on the
    ACT HWDGE ring (nc.scalar).  With both on one ring, chunk i+1's
    input DMA queues FIFO behind chunk i's output DMA, which waits on
    compute -> the DMA engines idle.  Separate rings let the input
    stream run back-to-back at HBM rate.
  - output is written fp16 (halves write traffic; tolerance is 2e-2,
    fp16 quantization is ~5e-4) and converted to f32 on host.
  - small last chunk shortens the pipeline drain tail.
"""

import numpy as np

import concourse.bass as bass
import concourse.mybir as mybir
from concourse.tile import TileContext
from concourse.bass_utils import run_bass_kernel_spmd

B = 4194304
NCORES = 8
P = 128
N_C = B // NCORES        # 524288 matrices per core
MPP = N_C // P           # 4096 matrices per partition
MS = [1024, 1024, 1024, 896, 128]   # per-chunk matrices per partition
assert sum(MS) == MPP

# component order in DRAM (flat 3x3 index): minuends, subtrahends, diagonal
PERM = [7, 2, 3, 5, 6, 1, 0, 4, 8]

F32 = mybir.dt.float32
F16 = mybir.dt.float16
ACT = mybir.ActivationFunctionType
ALU = mybir.AluOpType
PI_4 = float(np.pi / 4.0)
MAXM = max(MS)


def _split_multi_waits(nc):
    """This container's walrus build rejects >1 sem-wait per instruction
    ("Too many sync wait commands"); hoist extras onto preceding NOPs."""
    for f in nc.m.functions:
        for blk in f.blocks:
            il = blk.instructions
            new = []
            for ins in il:
                si = ins.sync_info
                if si is not None and si.on_wait is not None and len(si.on_wait) > 1:
                    waits = list(si.on_wait)
                    for j, w in enumerate(waits[:-1]):
                        nop = mybir.InstNoOp(name=f"{ins.name}-ws{j}", engine=ins.engine)
                        nop.sync_info = mybir.SyncInfo(on_wait=[w], on_update=[])
                        new.append(nop)
                    ins.sync_info = mybir.SyncInfo(
                        on_wait=[waits[-1]], on_update=list(si.on_update or [])
                    )
                new.append(ins)
            il[:] = new


def _build_kernel():
    nc = bass.Bass()
    # flat per-core buffers; chunk ci occupies rows [off*P*9 ...] tile-major
    x_in = nc.dram_tensor("mat_in", [N_C * 9], F32, kind="ExternalInput")
    y_out = nc.dram_tensor("twist_out", [N_C * 3], F16, kind="ExternalOutput")

    with TileContext(nc) as tc:
        with tc.tile_pool(name="io", bufs=2) as io_pool, \
             tc.tile_pool(name="io_out", bufs=3) as oo_pool, \
             tc.tile_pool(name="wp", bufs=2) as w_pool, \
             tc.tile_pool(name="tmp", bufs=2) as tmp:

            def do_chunk(ci, off, m):
                tile = io_pool.tile([P, 9 * MAXM], F32, tag="in", name=f"in{ci}")[:, : 9 * m]
                src = x_in[off * P * 9 : (off + m) * P * 9].rearrange(
                    "(p n) -> p n", p=P
                )
                nc.sync.dma_start(out=tile, in_=src)

                # w = minuends - subtrahends  (3m wide) -- on GpSimd to keep
                # the DVE under the per-chunk DMA period
                w = w_pool.tile([P, 3 * MAXM], F32, tag="w", name=f"w{ci}")[:, : 3 * m]
                nc.gpsimd.tensor_sub(
                    out=w, in0=tile[:, 0 : 3 * m], in1=tile[:, 3 * m : 6 * m]
                )

                tr = tmp.tile([P, MAXM], F32, tag="tr", name=f"tr{ci}")[:, :m]
                nc.vector.tensor_add(
                    out=tr, in0=tile[:, 6 * m : 7 * m], in1=tile[:, 7 * m : 8 * m]
                )
                nc.vector.tensor_add(out=tr, in0=tr, in1=tile[:, 8 * m : 9 * m])

                # x = cos(theta) = (tr-1)/2 -- on GpSimd (idle capacity)
                x = tmp.tile([P, MAXM], F32, tag="x", name=f"x{ci}")[:, :m]
                nc.gpsimd.tensor_scalar(
                    out=x, in0=tr, scalar1=0.5, scalar2=-0.5,
                    op0=ALU.mult, op1=ALU.add,
                )

                v = tmp.tile([P, MAXM], F32, tag="v", name=f"v{ci}")[:, :m]
                nc.scalar.activation(v, x, ACT.Square)
                lg = tmp.tile([P, MAXM], F32, tag="lg", name=f"lg{ci}")[:, :m]
                nc.scalar.activation(lg, v, ACT.Ln, bias=1.0, scale=-1.0)
                r = tmp.tile([P, MAXM], F32, tag="r", name=f"r{ci}")[:, :m]  # 1/sin(theta)
                nc.scalar.activation(r, lg, ACT.Exp, scale=-0.5)

                # xr = x*r = cot(theta)
                xr = tmp.tile([P, MAXM], F32, tag="xr", name=f"xr{ci}")[:, :m]
                nc.vector.tensor_mul(out=xr, in0=x, in1=r)
                t_at = tmp.tile([P, MAXM], F32, tag="t_at", name=f"t_at{ci}")[:, :m]
                nc.scalar.activation(t_at, xr, ACT.Arctan)

                # sc = (pi/4 - t_at/2) * r
                g = tmp.tile([P, MAXM], F32, tag="g", name=f"g{ci}")[:, :m]
                nc.vector.tensor_scalar(
                    out=g, in0=t_at, scalar1=-0.5, scalar2=PI_4,
                    op0=ALU.mult, op1=ALU.add,
                )
                sc = tmp.tile([P, MAXM], F32, tag="sc", name=f"sc{ci}")[:, :m]
                nc.vector.tensor_mul(out=sc, in0=g, in1=r)

                ot = oo_pool.tile([P, 3 * MAXM], F16, tag="out", name=f"out{ci}")[:, : 3 * m]
                for k in range(3):
                    nc.vector.tensor_mul(
                        out=ot[:, k * m : (k + 1) * m],
                        in0=sc,
                        in1=w[:, k * m : (k + 1) * m],
                    )
                dst = y_out[off * P * 3 : (off + m) * P * 3].rearrange(
                    "(p n) -> p n", p=P
                )
                nc.scalar.dma_start(out=dst, in_=ot)

            offs = np.concatenate([[0], np.cumsum(MS)[:-1]])
            for cj in range(len(MS)):
                do_chunk(cj, int(offs[cj]), MS[cj])

    _split_multi_waits(nc)
    return nc


_NC_CACHE = []


def _host_pack(mat_batch: np.ndarray) -> np.ndarray:
    """[B,3,3] -> [NCORES, N_C*9] tile-major/component-major PERM layout."""
    flat = np.ascontiguousarray(mat_batch, dtype=np.float32).reshape(
        NCORES, N_C, 9
    )
    out = np.empty((NCORES, N_C * 9), np.float32)
    pos = 0
    for m, off in zip(MS, np.concatenate([[0], np.cumsum(MS)[:-1]])):
        off = int(off)
        # chunk: matrices [off*P, (off+m)*P) viewed [P, m, 9] ->  [P, 9, m]
        chunk = flat[:, off * P : (off + m) * P, :].reshape(NCORES, P, m, 9)
        sz = P * m * 9
        out[:, pos : pos + sz] = (
            chunk.transpose(0, 1, 3, 2)[:, :, PERM, :].reshape(NCORES, sz)
        )
        pos += sz
    return out


def _host_unpack(res_list) -> np.ndarray:
    out = np.empty((B, 3), np.float32)
    o = out.reshape(NCORES, N_C, 3)
    for i, r in enumerate(res_list):
        y = r["twist_out"]
        pos = 0
        for m, off in zip(MS, np.concatenate([[0], np.cumsum(MS)[:-1]])):
            off = int(off)
            sz = P * m * 3
            blk = y[pos : pos + sz].reshape(P, 3, m)
            o[i, off * P : (off + m) * P, :] = blk.transpose(0, 2, 1).reshape(
                P * m, 3
            )
            pos += sz
    return out


def kernel(mat_batch: np.ndarray) -> np.ndarray:
    if not _NC_CACHE:
        _NC_CACHE.append(_build_kernel())
    nc = _NC_CACHE[0]

    packed = _host_pack(mat_batch)
    in_maps = [{"mat_in": packed[i]} for i in range(NCORES)]
    res = run_bass_kernel_spmd(nc, in_maps, core_ids=list(range(NCORES)))
    return _host_unpack(res.results)


# revision 5
# speedup vs baseline: 1.3209x; 1.3209x over previous
"""Mat2Twist Trainium2 kernel: batch of 3x3 rotation matrices -> twist vectors.

For each matrix R:  tr = trace(R); x = (tr-1)/2 = cos(theta)
  theta = arccos(x) = pi/2 - arctan(x / sqrt(1 - x^2))
  w = [R21-R12, R02-R20, R10-R01]   (unnormalized axis, |w| = 2 sin theta)
  out = theta * w / (2 sin theta) = (pi/4 - arctan(x*r)/2) * r * w,
        r = 1/sqrt(1-x^2) = exp(-0.5*ln(1-x^2)) = 1/sin(theta)

Data-parallel over 8 NeuronCores; memory-bound (~358 GB/s HBM per core),
so the design minimizes HBM bytes and keeps the input DMA stream dense:

  - mixed-precision staging: the 6 off-diagonal components (which only
    feed w) are packed fp16 on host; the 3 diagonal components stay f32
    (trace -> theta is ill-conditioned near theta=pi, w is not: its
    error enters the output as theta*r*dw/2 <= ~0.015 << the 2e-2
    tolerance).  Input drops 18.9 -> 12.6 MB/core.
  - output fp16 (3.1 MB/core), converted to f32 on host.
  - input DMAs on the SP HWDGE ring (nc.sync), outputs on the ACT ring
    (nc.scalar), so input DMAs never queue behind compute-dependent
    output DMAs.
  - all elementwise work on DVE (GpSimd shares SBUF ports with DVE and
    slows it down); the wide ops (w-sub, final scale muls) run in fp16
    for DVE's 2x 16-bit mode.

Host packs each core's shard tile-major/component-major: chunk ci covers
MS[ci] matrices per partition; within a partition-row components are
contiguous m-wide blocks, so every op and DMA is unit-stride.  Output is
produced component-major per chunk and re-interleaved on host.
"""

import numpy as np

import concourse.bass as bass
import concourse.mybir as mybir
from concourse.tile import TileContext
from concourse.bass_utils import run_bass_kernel_spmd

B = 4194304
NCORES = 8
P = 128
N_C = B // NCORES        # 524288 matrices per core
MPP = N_C // P           # 4096 matrices per partition
MS = [1024, 1024, 1024, 896, 128]   # per-chunk matrices per partition
assert sum(MS) == MPP

# off-diagonal flat 3x3 indices: minuends then subtrahends (w = a - b)
PERM16 = [7, 2, 3, 5, 6, 1]
# diagonal
PERM32 = [0, 4, 8]

F32 = mybir.dt.float32
F16 = mybir.dt.float16
ACT = mybir.ActivationFunctionType
ALU = mybir.AluOpType
PI_4 = float(np.pi / 4.0)
MAXM = max(MS)


def _split_multi_waits(nc):
    """This container's walrus build rejects >1 sem-wait per instruction
    ("Too many sync wait commands"); hoist extras onto preceding NOPs."""
    for f in nc.m.functions:
        for blk in f.blocks:
            il = blk.instructions
            new = []
            for ins in il:
                si = ins.sync_info
                if si is not None and si.on_wait is not None and len(si.on_wait) > 1:
                    waits = list(si.on_wait)
                    for j, w in enumerate(waits[:-1]):
                        nop = mybir.InstNoOp(name=f"{ins.name}-ws{j}", engine=ins.engine)
                        nop.sync_info = mybir.SyncInfo(on_wait=[w], on_update=[])
                        new.append(nop)
                    ins.sync_info = mybir.SyncInfo(
                        on_wait=[waits[-1]], on_update=list(si.on_update or [])
                    )
                new.append(ins)
            il[:] = new


def _build_kernel():
    nc = bass.Bass()
    # const AP for activation bias=-0.5 (only 0.0/1.0 pre-registered);
    # same registration pattern Bass.__init__ uses
    cm5 = nc.alloc_sbuf_tensor("const-float32-m0.5", [128, 1], F32)
    nc.gpsimd.memset(cm5.ap(), -0.5)
    nc.const_aps.aps[(F32, -0.5)] = cm5.ap()
    nc.all_engine_barrier()

    x16 = nc.dram_tensor("mat16", [N_C * 6], F16, kind="ExternalInput")
    x32 = nc.dram_tensor("mat32", [N_C * 3], F32, kind="ExternalInput")
    y_out = nc.dram_tensor("twist_out", [N_C * 3], F16, kind="ExternalOutput")

    with TileContext(nc) as tc:
        with tc.tile_pool(name="io16", bufs=3) as i16_pool, \
             tc.tile_pool(name="io32", bufs=3) as i32_pool, \
             tc.tile_pool(name="io_out", bufs=3) as oo_pool, \
             tc.tile_pool(name="wp", bufs=2) as w_pool, \
             tc.tile_pool(name="tmp", bufs=2) as tmp:

            def do_chunk(ci, off, m):
                t16 = i16_pool.tile([P, 6 * MAXM], F16, tag="in16", name=f"i16_{ci}")[:, : 6 * m]
                nc.sync.dma_start(
                    out=t16,
                    in_=x16[off * P * 6 : (off + m) * P * 6].rearrange("(p n) -> p n", p=P),
                )
                t32 = i32_pool.tile([P, 3 * MAXM], F32, tag="in32", name=f"i32_{ci}")[:, : 3 * m]
                nc.sync.dma_start(
                    out=t32,
                    in_=x32[off * P * 3 : (off + m) * P * 3].rearrange("(p n) -> p n", p=P),
                )

                # w = minuends - subtrahends, fp16 (DVE 2x 16-bit mode)
                w = w_pool.tile([P, 3 * MAXM], F16, tag="w", name=f"w{ci}")[:, : 3 * m]
                nc.vector.tensor_sub(
                    out=w, in0=t16[:, 0 : 3 * m], in1=t16[:, 3 * m : 6 * m]
                )

                tr = tmp.tile([P, MAXM], F32, tag="tr", name=f"tr{ci}")[:, :m]
                nc.vector.tensor_add(
                    out=tr, in0=t32[:, 0:m], in1=t32[:, m : 2 * m]
                )
                nc.vector.tensor_add(out=tr, in0=tr, in1=t32[:, 2 * m : 3 * m])

                # v = x^2 = (0.5*tr - 0.5)^2
                v = tmp.tile([P, MAXM], F32, tag="v", name=f"v{ci}")[:, :m]
                nc.scalar.activation(v, tr, ACT.Square, bias=-0.5, scale=0.5)
                lg = tmp.tile([P, MAXM], F32, tag="lg", name=f"lg{ci}")[:, :m]
                nc.scalar.activation(lg, v, ACT.Ln, bias=1.0, scale=-1.0)
                r = tmp.tile([P, MAXM], F32, tag="r", name=f"r{ci}")[:, :m]  # 1/sin(theta)
                nc.scalar.activation(r, lg, ACT.Exp, scale=-0.5)

                # xr2 = (tr-1)*r = 2*cot(theta); Arctan applies the 0.5
                xr = tmp.tile([P, MAXM], F32, tag="xr", name=f"xr{ci}")[:, :m]
                nc.vector.scalar_tensor_tensor(
                    out=xr, in0=tr, scalar=-1.0, in1=r, op0=ALU.add, op1=ALU.mult
                )
                t_at = tmp.tile([P, MAXM], F32, tag="t_at", name=f"t_at{ci}")[:, :m]
                nc.scalar.activation(t_at, xr, ACT.Arctan, scale=0.5)

                # sc = (pi/4 - t_at/2) * r, stored fp16 for the 2x muls
                g = tmp.tile([P, MAXM], F32, tag="g", name=f"g{ci}")[:, :m]
                nc.vector.tensor_scalar(
                    out=g, in0=t_at, scalar1=-0.5, scalar2=PI_4,
                    op0=ALU.mult, op1=ALU.add,
                )
                sc = tmp.tile([P, MAXM], F16, tag="sc", name=f"sc{ci}")[:, :m]
                nc.vector.tensor_mul(out=sc, in0=g, in1=r)

                ot = oo_pool.tile([P, 3 * MAXM], F16, tag="out", name=f"out{ci}")[:, : 3 * m]
                for k in range(3):
                    nc.vector.tensor_mul(
                        out=ot[:, k * m : (k + 1) * m],
                        in0=sc,
                        in1=w[:, k * m : (k + 1) * m],
                    )
                nc.scalar.dma_start(
                    out=y_out[off * P * 3 : (off + m) * P * 3].rearrange("(p n) -> p n", p=P),
                    in_=ot,
                )

            offs = np.concatenate([[0], np.cumsum(MS)[:-1]])
            for cj in range(len(MS)):
                do_chunk(cj, int(offs[cj]), MS[cj])

    _split_multi_waits(nc)
    return nc


_NC_CACHE = []


def _host_pack(mat_batch: np.ndarray) -> dict:
    """[B,3,3] -> {"mat16": [NCORES, N_C*6] f16, "mat32": [NCORES, N_C*3] f32}
    tile-major/component-major chunk layout."""
    flat = np.ascontiguousarray(mat_batch, dtype=np.float32).reshape(
        NCORES, N_C, 9
    )
    o16 = np.empty((NCORES, N_C * 6), np.float16)
    o32 = np.empty((NCORES, N_C * 3), np.float32)
    p16 = p32 = 0
    for m, off in zip(MS, np.concatenate([[0], np.cumsum(MS)[:-1]])):
        off = int(off)
        chunk = flat[:, off * P : (off + m) * P, :].reshape(NCORES, P, m, 9)
        cT = chunk.transpose(0, 1, 3, 2)  # [NC, P, 9, m]
        s16 = P * m * 6
        s32 = P * m * 3
        o16[:, p16 : p16 + s16] = cT[:, :, PERM16, :].reshape(NCORES, s16)
        o32[:, p32 : p32 + s32] = cT[:, :, PERM32, :].reshape(NCORES, s32)
        p16 += s16
        p32 += s32
    return {"mat16": o16, "mat32": o32}


def _host_unpack(res_list) -> np.ndarray:
    out = np.empty((B, 3), np.float32)
    o = out.reshape(NCORES, N_C, 3)
    for i, r in enumerate(res_list):
        y = r["twist_out"]
        pos = 0
        for m, off in zip(MS, np.concatenate([[0], np.cumsum(MS)[:-1]])):
            off = int(off)
            sz = P * m * 3
            blk = y[pos : pos + sz].reshape(P, 3, m)
            o[i, off * P : (off + m) * P, :] = blk.transpose(0, 2, 1).reshape(
                P * m, 3
            )
            pos += sz
    return out


def kernel(mat_batch: np.ndarray) -> np.ndarray:
    if not _NC_CACHE:
        _NC_CACHE.append(_build_kernel())
    nc = _NC_CACHE[0]

    packed = _host_pack(mat_batch)
    in_maps = [
        {name: arr[i] for name, arr in packed.items()} for i in range(NCORES)
    ]
    res = run_bass_kernel_spmd(nc, in_maps, core_ids=list(range(NCORES)))
    return _host_unpack(res.results)
